# revision 32
# baseline (speedup 1.0000x reference)
"""Multi-head self-attention (RMSNorm + causal MHA + out-proj) on 8 TRN2 cores.

Sharding (tensor-parallel per the hint): core c handles batch b = c//4 and
head group hg = c%4 (4 of 16 heads). Each core computes a PARTIAL output (its
heads' slice of the out-projection contraction); the host sums the 4 partials
per batch — the reduce inherent in head-split TP — and transposes back.

Device kernel (per core, feature-major / transposed orientation throughout so
no on-chip transposes are ever needed):
  - RMSNorm via an all-ones 128x128 matmul (partition reduce + broadcast to
    all partitions in one shot); rstd in ONE fused Rsqrt activation.
  - bf16 compute on TensorE. Norm weight folded into the projection weights
    on the host (exact); weights shipped pre-transposed bf16.
  - Heads processed in PAIRS: the QKV projection is laid out so head A of a
    pair lands on partitions 0-63 and head B on 64-127. The two heads' score
    matmuls (contract dim d=64) then row-tile the PE array concurrently with
    no data duplication.
  - Causal flash attention computed transposed: S^T = K^T.T @ Q^T. exp on
    ScalarE with fused 1/sqrt(d) scale, restricted to the causal column
    window on diagonal tiles; no max-subtraction (scores ~N(0,1)).
  - Softmax denominator fused into the PV matmul via a ones column appended
    to V (M=65); ctx^T and l accumulate together in PSUM for both heads of a
    pair (one [65,1024] PSUM tile, A cols 0:512 / B cols 512:1024).
  - ctx PSUM evacuated with a single fast copy (releases PSUM); 1/l is then
    broadcast across partitions with a GpSimd partition_broadcast (no DRAM
    bounce) and applied on VectorE. Head-B halves reach partitions 64-127
    via a Pool-issued SBUF-SBUF DMA.
  - Out-projection consumes ctx^T; stores batched (one DMA per 512-chunk,
    bf16); host sums partial outputs in fp32.
"""

from contextlib import ExitStack

import numpy as np
import ml_dtypes

import concourse.bass as bass
import concourse.tile as tile
from concourse import bacc, mybir
from concourse.bass_utils import run_bass_kernel_spmd

F32 = mybir.dt.float32
BF16 = mybir.dt.bfloat16
AF = mybir.ActivationFunctionType
P = 128
DD = 64
T = 2048
D = 1024
NH = 4            # heads per core
NP = 2            # head pairs per core
N_CORES = 8
EPS = 1e-6


def build_kernel(nc, reps=1):
    KT = D // P           # 8 contraction tiles for projections
    TT = T // P           # 16 token tiles
    TC = T // 512         # 4 query chunks
    QK = 4 * P            # qkv_w column count for q+k (2 pairs x (q128|k128))
    VF = NH * DD          # 256 v features

    xT_d = nc.dram_tensor("xT", [D, T], BF16, kind="ExternalInput")
    wqkT_d = nc.dram_tensor("wqkT", [D, QK], BF16, kind="ExternalInput")
    wvT_d = nc.dram_tensor("wvT", [D, VF], BF16, kind="ExternalInput")
    woT_d = nc.dram_tensor("woT", [VF, D], BF16, kind="ExternalInput")
    outT_d = nc.dram_tensor("outT", [D, T], BF16, kind="ExternalOutput")

    with tile.TileContext(nc) as tc, ExitStack() as ctx:
        consts = ctx.enter_context(tc.tile_pool(name="consts", bufs=1))
        persist = ctx.enter_context(tc.tile_pool(name="persist", bufs=1))
        xsqp = ctx.enter_context(tc.tile_pool(name="xsqp", bufs=3))
        epool = ctx.enter_context(tc.tile_pool(name="epool", bufs=6))
        cvp = ctx.enter_context(tc.tile_pool(name="cvp", bufs=3))
        rlp = ctx.enter_context(tc.tile_pool(name="rlp", bufs=3))
        osbp = ctx.enter_context(tc.tile_pool(name="osbp", bufs=2))
        # PSUM: ONE shared 3-deep pool of [128,1024] tiles (6 banks) for
        # scores AND all projection groups — 3 bufs keeps the PE queue from
        # head-blocking on exp (HW-measured 10x) — plus ctx accumulator (2).
        sps = ctx.enter_context(tc.tile_pool(name="sps", bufs=3, space="PSUM"))
        ctxp = ctx.enter_context(tc.tile_pool(name="ctxp", bufs=1, space="PSUM"))

        def emit_body(iv=None):
            ones_bf = consts.tile([P, P], BF16)
            nc.vector.memset(ones_bf[:], 1.0)
            eps_sb = consts.tile([P, 1], F32)
            nc.vector.memset(eps_sb[:], EPS)
            mask_bf = consts.tile([P, P], BF16)
            nc.gpsimd.memset(mask_bf[:], 1.0)
            nc.gpsimd.affine_select(
                out=mask_bf[:], in_=mask_bf[:],
                compare_op=mybir.AluOpType.is_ge, fill=0.0, base=0,
                pattern=[[1, P]], channel_multiplier=-1,
            )

            # ---- weights + x, chunk-major so chunk-0 RMSNorm and the first
            # projections start ~4us in instead of waiting for the full x ----
            xbf = persist.tile([P, KT, T], BF16)
            wqk_bf = persist.tile([P, KT, QK], BF16)
            wv_bf = persist.tile([P, KT, VF], BF16)
            wo_bf = persist.tile([P, VF // P, D], BF16)
            for kt in range(KT):
                nc.sync.dma_start(
                    xbf[:, kt, 0:512], xT_d.ap()[kt * P : (kt + 1) * P, 0:512]
                )
            for kt in range(KT):
                nc.sync.dma_start(wqk_bf[:, kt, :], wqkT_d.ap()[kt * P : (kt + 1) * P, :])
            for c in range(1, TC):
                cs = slice(512 * c, 512 * (c + 1))
                for kt in range(KT):
                    nc.sync.dma_start(xbf[:, kt, cs], xT_d.ap()[kt * P : (kt + 1) * P, cs])
                if c == 1:
                    for kt in range(KT):
                        nc.sync.dma_start(
                            wv_bf[:, kt, :], wvT_d.ap()[kt * P : (kt + 1) * P, :]
                        )
                if c == 2:
                    for ct in range(VF // P):
                        nc.sync.dma_start(
                            wo_bf[:, ct, :], woT_d.ap()[ct * P : (ct + 1) * P, :]
                        )

            # ---- RMSNorm, per 512-chunk --------------------------------
            rstd_bf = persist.tile([P, T], BF16)
            xn = persist.tile([P, KT, T], BF16)

            def emit_rms_chunk(c):
                cs = slice(512 * c, 512 * (c + 1))
                mst = sps.tile([P, 1024], F32, tag="sst")
                ms = mst[:, 0:512]
                eng = nc.vector if c == 0 else nc.gpsimd  # c0 is latency-critical
                for kt in range(KT):
                    xsq = xsqp.tile([P, 512], BF16, tag="xsq")
                    eng.tensor_mul(xsq[:], xbf[:, kt, cs], xbf[:, kt, cs])
                    nc.tensor.matmul(
                        ms, ones_bf[:], xsq[:],
                        start=(kt == 0), stop=(kt == KT - 1),
                    )
                sq = xsqp.tile([P, 512], F32, tag="sq")
                nc.scalar.activation(
                    sq[:], ms, AF.Sqrt, bias=eps_sb[:, 0:1], scale=1.0 / D,
                )
                with nc.allow_low_precision(reason="rstd feeds bf16 matmuls"):
                    nc.vector.reciprocal(rstd_bf[:, cs], sq[:])
                for kt in range(KT):
                    nc.vector.tensor_mul(xn[:, kt, cs], xbf[:, kt, cs], rstd_bf[:, cs])

            # Q^T/K^T per pair: [hA on parts 0:64 | hB on parts 64:128, T]
            QTd = persist.tile([P, NP, T], BF16)
            KTd = persist.tile([P, NP, T], BF16)
            # V token-major, per tile: 4 x (64 feats + ones col)
            Vsb = persist.tile([P, TT, 65 * NH], BF16)
            nc.vector.memset(
                Vsb[:].rearrange("p t (h c) -> p t h c", h=NH)[:, :, :, DD : DD + 1],
                1.0,
            )
            # normalized ctx^T: pair tile = [A feats 0:64 | B feats 64:128, T]
            ctxn = persist.tile([P, NP, T], BF16)

            def emit_qk_pair(p, c):
                """Project q AND k features of pair p for 512-token chunk c
                into the two halves of one PSUM tile; one evac per half."""
                qkps = sps.tile([P, 1024], F32, tag="sst")
                for is_k in range(2):
                    wcol = 256 * p + 128 * is_k
                    half = qkps[:, 512 * is_k : 512 * (is_k + 1)]
                    for kt in range(KT):
                        nc.tensor.matmul(
                            half,
                            wqk_bf[:, kt, wcol : wcol + P],
                            xn[:, kt, 512 * c : 512 * (c + 1)],
                            start=(kt == 0), stop=(kt == KT - 1),
                        )
                cs = slice(512 * c, 512 * (c + 1))
                nc.vector.tensor_copy(QTd[:, p, cs], qkps[:, 0:512])
                nc.vector.tensor_copy(KTd[:, p, cs], qkps[:, 512:1024])

            def emit_v_t2(t2):
                vps = sps.tile([P, 1024], F32, tag="sst")
                for u in range(2):
                    tt = 2 * t2 + u
                    half = vps[:, 512 * u : 512 * u + VF]
                    for kt in range(KT):
                        nc.tensor.matmul(
                            half,
                            xn[:, kt, P * tt : P * (tt + 1)],
                            wv_bf[:, kt, :],
                            start=(kt == 0), stop=(kt == KT - 1),
                        )
                nc.vector.tensor_copy(
                    Vsb[:, 2 * t2 : 2 * t2 + 2, :]
                    .rearrange("p t (h c) -> p t h c", h=NH)[:, :, :, 0:DD],
                    vps[:]
                    .rearrange("p (t c) -> p t c", t=2)[:, :, 0:VF]
                    .rearrange("p t (h c) -> p t h c", h=NH),
                )

            def emit_attn_chunk(p, c, fillers=None):
                """Attention for head pair p over query chunk c (both heads
                concurrently: A via PE rows 0-63, B via rows 64-127).
                `fillers` are projection-group closures interleaved between
                j-steps so PE GEMMs overlap the ACT exp stream."""
                hA, hB = 2 * p, 2 * p + 1
                ctx_ps = ctxp.tile([65, 1024], F32, tag="ctx")
                njt = 4 * (c + 1)
                qs = 512 * c
                for j in range(njt):
                    # pop a filler every 3rd j — never starve the 3-deep
                    # score-PSUM pipeline (2-deep head-blocks the PE queue)
                    if fillers and j % 3 == 2:
                        fillers.popleft()()
                    off = max(0, 128 * j - qs)
                    w = 512 - off
                    # B packed right after A so exp runs as ONE contiguous op
                    bA, bB = off, 512
                    sst = sps.tile([P, 1024], F32, tag="sst")
                    nc.tensor.matmul(
                        sst[:, bA : bA + w],
                        KTd[0:DD, p, P * j : P * (j + 1)],
                        QTd[0:DD, p, qs + off : qs + 512],
                        start=True, stop=True,
                    )
                    nc.tensor.matmul(
                        sst[:, bB : bB + w],
                        KTd[DD:P, p, P * j : P * (j + 1)],
                        QTd[DD:P, p, qs + off : qs + 512],
                        start=True, stop=True,
                    )
                    expS = epool.tile([P, 1024], BF16, tag="e")
                    nc.scalar.activation(
                        expS[:, bA : bB + w], sst[:, bA : bB + w], AF.Exp, scale=0.125
                    )
                    if 128 * j >= qs:  # diagonal tile: zero above-diagonal
                        nc.vector.tensor_mul(
                            expS[:, bA : bA + P], expS[:, bA : bA + P], mask_bf[:]
                        )
                        nc.vector.tensor_mul(
                            expS[:, bB : bB + P], expS[:, bB : bB + P], mask_bf[:]
                        )
                    nc.tensor.matmul(
                        ctx_ps[:, off:512],
                        Vsb[:, j, 65 * hA : 65 * (hA + 1)],
                        expS[:, bA : bA + w],
                        start=(j == 0), stop=(j == njt - 1),
                    )
                    nc.tensor.matmul(
                        ctx_ps[:, 512 + off : 1024],
                        Vsb[:, j, 65 * hB : 65 * (hB + 1)],
                        expS[:, bB : bB + w],
                        start=(j == 0), stop=(j == njt - 1),
                    )
                while fillers:
                    fillers.popleft()()
                return ctx_ps

            def emit_norm(p, c, ctx_ps):
                """Normalize pair ctx by 1/l; land A on parts 0:64 directly,
                B via a Pool-issued partition-shifting SBUF DMA."""
                cs = slice(512 * c, 512 * (c + 1))
                cv = cvp.tile([65, 1024], BF16, tag="cv")
                nc.vector.tensor_copy(cv[:], ctx_ps[:])  # releases PSUM
                lsb = rlp.tile([65, 1024], BF16, tag="lsb")
                with nc.allow_low_precision(reason="softmax denom in bf16"):
                    nc.vector.reciprocal(lsb[DD : DD + 1, :], cv[DD : DD + 1, :])
                # broadcast 1/l across partitions via a K=1 outer-product
                # matmul (ones column x row): no DMA, lands in PSUM
                rlt = sps.tile([P, 1024], F32, tag="sst")
                for s in range(2):
                    nc.tensor.matmul(
                        rlt[0:DD, 512 * s : 512 * (s + 1)],
                        ones_bf[DD : DD + 1, 0:DD],
                        lsb[DD : DD + 1, 512 * s : 512 * (s + 1)],
                        start=True, stop=True,
                    )
                nc.vector.tensor_mul(ctxn[0:DD, p, cs], cv[0:DD, 0:512], rlt[0:DD, 0:512])
                tmpb = rlp.tile([DD, 512], BF16, tag="tmpb")
                nc.vector.tensor_mul(tmpb[:], cv[0:DD, 512:1024], rlt[0:DD, 512:1024])
                nc.sync.dma_start(ctxn[DD:P, p, cs], tmpb[:])

            def emit_outproj_group(c, ep, state):
                if "osb" not in state:
                    state["osb"] = osbp.tile(
                        [P, D // P, 512], BF16, tag="osb", name="osb"
                    )
                osb = state["osb"]
                ops = sps.tile([P, 1024], F32, tag="sst")
                for s in range(2):
                    e = 2 * ep + s
                    half = ops[:, 512 * s : 512 * (s + 1)]
                    for ct in range(NP):
                        nc.tensor.matmul(
                            half,
                            wo_bf[:, ct, P * e : P * (e + 1)],
                            ctxn[:, ct, 512 * c : 512 * (c + 1)],
                            start=(ct == 0), stop=(ct == NP - 1),
                        )
                nc.vector.tensor_copy(
                    osb[:].rearrange("p e t -> p (e t)")[
                        :, 1024 * ep : 1024 * (ep + 1)
                    ],
                    ops[:],
                )
                if ep % 2 == 1:  # store in halves so the tail store is small
                    nc.sync.dma_start(
                        outT_d.ap().rearrange("(e p) t -> p e t", p=P)[
                            :, 2 * ep - 2 : 2 * ep + 2, 512 * c : 512 * (c + 1)
                        ],
                        osb[:, 2 * ep - 2 : 2 * ep + 2, :],
                    )

            def outproj_fillers(c):
                state = {}
                return [
                    (lambda ep=ep: emit_outproj_group(c, ep, state))
                    for ep in range(D // P // 2)
                ]

            # Wavefront over query chunks: QK proj chunk c (both pairs) + V
            # tiles 4c..4c+3 land just before the attention that needs them,
            # so projection GEMMs fill PE gaps while ACT churns exp.
            from collections import deque

            emit_rms_chunk(0)
            emit_qk_pair(0, 0)
            emit_v_t2(0)
            emit_v_t2(1)
            for c in range(TC):
                opf = outproj_fillers(c - 1) if c >= 1 else []
                # on the last wave, spread the outproj groups across BOTH
                # attention chunks so the long pair-1 chunk gets PE fillers
                fa = deque(opf[:2] if c == TC - 1 else opf)
                cps = emit_attn_chunk(0, c, fa)
                emit_qk_pair(1, c)
                emit_norm(0, c, cps)
                fb = deque(opf[2:] if c == TC - 1 else [])
                if c + 1 < TC:
                    fb.append(lambda c=c: emit_rms_chunk(c + 1))
                    fb.append(lambda c=c: emit_v_t2(2 * c + 2))
                    fb.append(lambda c=c: emit_v_t2(2 * c + 3))
                    fb.append(lambda c=c: emit_qk_pair(0, c + 1))
                cps = emit_attn_chunk(1, c, fb)
                emit_norm(1, c, cps)
            for f in outproj_fillers(TC - 1):
                f()

        if reps == 1:
            emit_body()
        else:
            with tc.For_i(0, reps, 1) as iv:
                emit_body(iv)


_NC_CACHE = None


def _get_nc():
    global _NC_CACHE
    if _NC_CACHE is None:
        nc = bacc.Bacc(
            "TRN2", target_bir_lowering=False, debug=False, num_devices=N_CORES
        )
        build_kernel(nc)
        nc.compile()
        _NC_CACHE = nc
    return _NC_CACHE


def make_in_maps(x, norm_weight, qkv_w, out_w):
    x = np.asarray(x, dtype=np.float32)
    norm_weight = np.asarray(norm_weight, dtype=np.float32)
    qkv_w = np.asarray(qkv_w, dtype=np.float32)
    out_w = np.asarray(out_w, dtype=np.float32)
    # fold the RMSNorm weight into the projection weights (exact in fp32)
    qkv_eff = qkv_w * norm_weight[None, :]
    bf = ml_dtypes.bfloat16
    in_maps = []
    for core in range(N_CORES):
        b, hg = core // 4, core % 4
        r0 = 256 * hg
        xT = np.ascontiguousarray(x[b].T.astype(bf))
        # wqk columns: [pair0: qA qB | kA kB, pair1: qC qD | kC kD]
        blocks = []
        for p in range(2):
            blocks.append(qkv_eff[r0 + 128 * p : r0 + 128 * p + 128])          # q pair
            blocks.append(qkv_eff[D + r0 + 128 * p : D + r0 + 128 * p + 128])  # k pair
        wqkT = np.ascontiguousarray(np.concatenate(blocks, 0).T.astype(bf))
        wvT = np.ascontiguousarray(qkv_eff[2 * D + r0 : 2 * D + r0 + 256].T.astype(bf))
        woT = np.ascontiguousarray(out_w[:, r0 : r0 + 256].T.astype(bf))
        in_maps.append({"xT": xT, "wqkT": wqkT, "wvT": wvT, "woT": woT})
    return in_maps


def gather_output(results):
    out = np.empty((2, T, D), np.float32)
    for b in range(2):
        acc = results[4 * b]["outT"].astype(np.float32)
        for hg in range(1, 4):
            acc = acc + results[4 * b + hg]["outT"].astype(np.float32)
        out[b] = acc.T
    return out


def kernel(x, norm_weight, qkv_w, out_w):
    nc = _get_nc()
    in_maps = make_in_maps(x, norm_weight, qkv_w, out_w)
    res = run_bass_kernel_spmd(nc, in_maps, core_ids=list(range(N_CORES)))
    return gather_output(res.results)
